# revision 31
# baseline (speedup 1.0000x reference)
"""Dilated attention (segment 64, dilation 4, 16 heads, head_dim 64) on 8 trn2 cores.

Sharding: 2 batches x 4 head-groups (4 heads each) = 8 cores. Each core computes
q/k/v projections for its 4 heads on its batch, block-sparse attention over the
+-2 block (256-token) dilated band, and a partial output projection. Host sums
the 4 head-group partials per batch.

Layout is fully "transposed" on-core to avoid PE transposes:
  xT   [D, S]    (D on partitions, 8 chunks of 128)
  qT/kT [64h, S] per head (head dim on partitions)
  v    [S, 64]   natural (keys on partitions) + ones column -> softmax denoms
  scoresT [k-block 128, q-window <=640] = kT_blk-stationary x qT-window
  exp (no max subtraction; scores are N(0,1)-ish), binary mask multiply,
  PV accumulates outT [65, S-half] per head in PSUM (row 64 = denominators),
  normalize via reciprocal + gpsimd partition_broadcast + DVE mul,
  yT [D, S] = wo-stationary x outT_norm, summed on host.

Scheduling strategy: attention is ACT(exp)-paced, so the k/v/q1/k1
projections are emitted BETWEEN attention stages at lower scheduler
priority — the PE fills attention dependency stalls with projection
matmuls, which also keeps the HAM clock-gate warm.
"""

import numpy as np
import ml_dtypes

bfloat16 = ml_dtypes.bfloat16

B, S, D = 2, 2048, 1024
H, Dh = 16, 64
NCORES = 8
NKB = S // 128  # 16 key blocks
WMAX = 640

_cache = {}


def _mask_rel():
    kp = np.arange(128)[:, None]
    j = np.arange(WMAX)[None, :]
    qrel = j - 256
    diff = np.abs(qrel - kp)
    seg = (qrel // 64) == (kp // 64)
    dil = (diff > 0) & (diff % 4 == 0) & (diff <= 256)
    return np.ascontiguousarray((seg | dil).astype(bfloat16))


def _win(kb):
    return max(0, kb - 2) * 128, min(NKB, kb + 3) * 128


def _pv_pieces(kb):
    """Split PV matmul for key-block kb at psum bank boundaries.

    PSUM start_tensor_calc zeroes the whole 2KB bank (zero-region), so
    start/stop are decided per BANK: the first key-block whose window
    touches a bank opens it (start=True zeroes it), the last closes it."""
    q0, q1 = _win(kb)
    out = []
    a = q0
    while a < q1:
        b = min(q1, (a // 512 + 1) * 512)
        bank = a // 512
        st = kb == max(0, 4 * bank - 2)
        sp = kb == min(NKB - 1, 4 * bank + 5)
        out.append((a, b, st, sp))
        a = b
    return q0, q1, out


def _build(debug=False):
    key = ("nc", debug)
    if key in _cache:
        return _cache[key]
    import concourse.mybir as mybir
    from concourse import bacc
    from concourse.tile import TileContext

    bf = mybir.dt.bfloat16
    f32 = mybir.dt.float32
    EXP = mybir.ActivationFunctionType.Exp

    nc = bacc.Bacc()
    d_x = nc.declare_dram_parameter("xT", [128, 8, S], bf, isOutput=False)
    d_wq = nc.declare_dram_parameter("wq", [128, 8, 256], bf, isOutput=False)
    d_wk = nc.declare_dram_parameter("wk", [128, 8, 256], bf, isOutput=False)
    d_wv = nc.declare_dram_parameter("wv", [128, 8, 256], bf, isOutput=False)
    d_wo = nc.declare_dram_parameter("wo", [128, 2, 1024], bf, isOutput=False)
    d_mask = nc.declare_dram_parameter("maskT", [128, WMAX], bf, isOutput=False)
    f16 = mybir.dt.float16
    d_y = nc.declare_dram_parameter("yT", [128, 8, S], f16, isOutput=True)
    if debug:
        d_dbg_q = nc.declare_dram_parameter("dbg_q", [128, S], bf, isOutput=True)
        d_dbg_k = nc.declare_dram_parameter("dbg_k", [128, S], bf, isOutput=True)
        d_dbg_v = nc.declare_dram_parameter("dbg_v", [128, 16 * 4 * 65], bf, isOutput=True)
        d_dbg_at = nc.declare_dram_parameter("dbg_at", [128, WMAX], bf, isOutput=True)
        d_dbg_den = nc.declare_dram_parameter("dbg_den", [4, S], f32, isOutput=True)
        d_dbg_rec = nc.declare_dram_parameter("dbg_rec", [4, S], f32, isOutput=True)
        d_dbg_on = nc.declare_dram_parameter("dbg_on", [128, S], bf, isOutput=True)

    with TileContext(nc) as tc:
        with (
            tc.tile_pool(name="const", bufs=1) as cpool,
            tc.tile_pool(name="attn", bufs=68) as apool,
            tc.tile_pool(name="ysb", bufs=4) as ypool,
            tc.tile_pool(name="small", bufs=2) as spool,
        ):
            # ---- input DMAs, dispatch spread over sync/gpsimd/scalar ----
            sb_wq = cpool.tile([128, 8, 256], bf, name="wq", tag="wq")
            sb_wk = cpool.tile([128, 8, 256], bf, name="wk", tag="wk")
            sb_wv = cpool.tile([128, 8, 256], bf, name="wv", tag="wv")
            nc.scalar.dma_start(out=sb_wq[:, 0, :], in_=d_wq[:, 0, :])
            sb_x = []
            for dc in range(8):
                t = cpool.tile([128, S], bf, name=f"x{dc}", tag=f"x{dc}")
                nc.sync.dma_start(out=t[0:64, :], in_=d_x[0:64, dc, :])
                nc.gpsimd.dma_start(out=t[64:128, :], in_=d_x[64:128, dc, :])
                sb_x.append(t)
            nc.scalar.dma_start(out=sb_wq[:, 1:8, :], in_=d_wq[:, 1:8, :])
            nc.scalar.dma_start(out=sb_wk[:, :, :], in_=d_wk[:, :, :])
            nc.scalar.dma_start(out=sb_wv[:, :, :], in_=d_wv[:, :, :])
            sb_wo = cpool.tile([128, 2, 1024], bf, name="wo", tag="wo")
            nc.scalar.dma_start(out=sb_wo, in_=d_wo[:, :, :])
            sb_mask = cpool.tile([128, WMAX], bf, name="mask", tag="mask")
            nc.scalar.dma_start(out=sb_mask, in_=d_mask[:, :])

            sb_q = []
            sb_k = []
            sb_on = []
            for p in range(2):
                sb_q.append(cpool.tile([128, S], bf, name=f"q{p}", tag=f"q{p}"))
                sb_k.append(cpool.tile([128, S], bf, name=f"k{p}", tag=f"k{p}"))
                sb_on.append(cpool.tile([128, S], bf, name=f"on{p}", tag=f"on{p}"))
            sb_v = cpool.tile([128, 16, 4, 65], bf, name="v", tag="v")
            nc.vector.memset(sb_v[:, :, :, 64:65], 1.0)

            # ---- era 0: q0 + k0 projections, dc-outer accumulators.
            # Dep-free junk matmuls interleave into the dc chains so the HAM
            # busy-window stays fed during x-chunk DMA arrival gaps (else the
            # PE clock halves at ~14us and the whole era runs 2x slow).
            wpool = tc.alloc_tile_pool(name="warm", bufs=1, space="PSUM")
            junk = cpool.tile([128, 512], bf, name="junk", tag="junk")
            nc.vector.memset(junk, 0.0)
            wps = wpool.tile([128, 512], f32, name="wps", tag="wps")

            def junk_mm(n=1):
                for _ in range(n):
                    nc.tensor.matmul(wps, lhsT=junk[:, 0:128], rhs=junk,
                                     start=True, stop=True)

            with tc.tile_pool(name="pjq", bufs=1, space="PSUM") as pjq:
                junk_mm(12)
                for w_sb, dst, scope in (
                    (sb_wq, sb_q, "proj_q0"),
                    (sb_wk, sb_k, "proj_k0"),
                ):
                    with nc.named_scope(scope):
                        acc = {}
                        for tt in range(4):
                            acc[tt] = pjq.tile([128, 512], f32, name=f"a{tt}", tag=f"aq{tt}")
                        for dc in range(8):
                            for tt in range(4):
                                nc.tensor.matmul(
                                    acc[tt],
                                    lhsT=w_sb[:, dc, 0:128],
                                    rhs=sb_x[dc][:, tt * 512:(tt + 1) * 512],
                                    start=(dc == 0),
                                    stop=(dc == 7),
                                )
                            if scope == "proj_q0":
                                junk_mm(2)
                        for tt in range(4):
                            nc.vector.tensor_copy(dst[0][:, tt * 512:(tt + 1) * 512], acc[tt])
            wpool.release()

            # ---- attention era: sc(2x2) + ot(3x1) + pj(1x1) = 8 psum banks ----
            with (
                tc.tile_pool(name="sc", bufs=2, space="PSUM") as scp,
                tc.tile_pool(name="ot", bufs=3, space="PSUM") as otp,
                tc.tile_pool(name="pj", bufs=1, space="PSUM") as pj,
            ):
                def proj_qk(w_sb, dst, p, scope):
                    with nc.named_scope(scope):
                        for tt in range(4):
                            ps = pj.tile([128, 512], f32, name="pspj", tag="pj")
                            for dc in range(8):
                                nc.tensor.matmul(
                                    ps,
                                    lhsT=w_sb[:, dc, p * 128:(p + 1) * 128],
                                    rhs=sb_x[dc][:, tt * 512:(tt + 1) * 512],
                                    start=(dc == 0),
                                    stop=(dc == 7),
                                )
                            nc.vector.tensor_copy(dst[p][:, tt * 512:(tt + 1) * 512], ps)

                def proj_v():
                    with nc.named_scope("proj_v"):
                        for t in range(16):
                            ps = pj.tile([128, 256], f32, name="psv", tag="pj")
                            for dc in range(8):
                                nc.tensor.matmul(
                                    ps,
                                    lhsT=sb_x[dc][:, t * 128:(t + 1) * 128],
                                    rhs=sb_wv[:, dc, :],
                                    start=(dc == 0),
                                    stop=(dc == 7),
                                )
                            nc.vector.tensor_copy(
                                sb_v[:, t, :, 0:64],
                                ps.rearrange("p (h d) -> p h d", h=4),
                            )

                def scores_phase(p):
                    ats = {}
                    with nc.named_scope(f"scores_p{p}"):
                        for kb in range(NKB):
                            q0, q1 = _win(kb)
                            wk_ = q1 - q0
                            j0 = q0 - (kb - 2) * 128
                            # two sc tiles (distinct psum slots) so the two
                            # K=64 head matmuls pack into row-groups 0-1/2-3
                            # of the PE array and run concurrently
                            sc = [
                                scp.tile([128, WMAX], f32, name=f"sc{hh}", tag="sc")
                                for hh in range(2)
                            ]
                            a = 0
                            while a < wk_:
                                b = min(a + 512, wk_)
                                for hh in range(2):
                                    half = hh * 64
                                    nc.tensor.matmul(
                                        sc[hh][:, a:b],
                                        lhsT=sb_k[p][half:half + 64, kb * 128:(kb + 1) * 128],
                                        rhs=sb_q[p][half:half + 64, q0 + a:q0 + b],
                                        start=True,
                                        stop=True,
                                    )
                                a = b
                            for hh in range(2):
                                at = apool.tile([128, WMAX], bf, name="at", tag="at")
                                nc.scalar.activation(at[:, :wk_], sc[hh][:, :wk_], EXP)
                                nc.vector.tensor_mul(
                                    at[:, :wk_], at[:, :wk_], sb_mask[:, j0:j0 + wk_]
                                )
                                ats[hh, kb] = at
                                if debug and p == 0 and hh == 0 and kb == 8:
                                    nc.sync.dma_start(out=d_dbg_at[:, :], in_=at[:, :])
                    return ats

                def pv_phase(p, hhs, ats):
                    bank_tiles = {}

                    def normalize(hh, bk, ot):
                        h = 2 * p + hh
                        half = hh * 64
                        base = bk * 512
                        den = spool.tile([1, 512], f32, name="den", tag="den")
                        nc.scalar.copy(den, ot[64:65, :])
                        rec = spool.tile([1, 512], f32, name="rec", tag="rec")
                        nc.vector.reciprocal_approx_fast(rec, den)
                        bc = spool.tile([64, 512], f32, name="bc", tag="bc")
                        nc.gpsimd.partition_broadcast(bc, rec)
                        if debug:
                            nc.sync.dma_start(out=d_dbg_rec[h:h + 1, base:base + 512], in_=rec)
                        nc.vector.tensor_mul(
                            sb_on[p][half:half + 64, base:base + 512],
                            ot[0:64, :], bc,
                        )

                    with nc.named_scope(f"pv_p{p}"):
                        for kb in range(NKB):
                            q0, q1, pieces = _pv_pieces(kb)
                            for hh in hhs:
                                h = 2 * p + hh
                                at = ats[hh, kb]
                                vv = sb_v[:, kb, h, :]
                                for a, b, st, sp_ in pieces:
                                    bk = a // 512
                                    if st:
                                        bank_tiles[hh, bk] = otp.tile(
                                            [65, 512], f32, name=f"o{h}b{bk}", tag="outp"
                                        )
                                    nc.tensor.matmul(
                                        bank_tiles[hh, bk][:, a - bk * 512:b - bk * 512],
                                        lhsT=vv,
                                        rhs=at[:, a - q0:b - q0],
                                        start=st,
                                        stop=sp_,
                                    )
                                    if sp_:
                                        normalize(hh, bk, bank_tiles[hh, bk])

                # pair 0: scores (ACT-paced), v-proj fills PE stalls
                ats0 = scores_phase(0)
                proj_v()
                # pair-1 projections early so scores1 can pipeline during pv0
                proj_qk(sb_wq, sb_q, 1, "proj_q1")
                proj_qk(sb_wk, sb_k, 1, "proj_k1")
                pv_phase(0, (0, 1), ats0)
                ats1 = scores_phase(1)
                pv_phase(1, (0, 1), ats1)

                if debug:
                    nc.sync.dma_start(out=d_dbg_q[:, :], in_=sb_q[0][:, :])
                    nc.sync.dma_start(out=d_dbg_k[:, :], in_=sb_k[0][:, :])
                    nc.sync.dma_start(
                        out=d_dbg_v[:, :],
                        in_=sb_v.rearrange("p a b c -> p (a b c)"),
                    )
                    nc.sync.dma_start(out=d_dbg_on[:, :], in_=sb_on[0][:, :])

                # ---- output projection: yT = wo^T @ outT_norm ----
                # shares the ot psum pool so early tt-waves overlap the tail
                with nc.named_scope("proj_y"):
                    f16_ = mybir.dt.float16
                    for tt in range(4):
                        for dch in range(4):
                            ysb = ypool.tile([128, 1024], f16_, name="ysb", tag="ysb")
                            for sub in range(2):
                                dc = dch * 2 + sub
                                ps = otp.tile([128, 512], f32, name="psy", tag="outp")
                                for kc in range(2):
                                    nc.tensor.matmul(
                                        ps,
                                        lhsT=sb_wo[:, kc, dc * 128:(dc + 1) * 128],
                                        rhs=sb_on[kc][:, tt * 512:(tt + 1) * 512],
                                        start=(kc == 0),
                                        stop=(kc == 1),
                                    )
                                if sub == 0:
                                    nc.scalar.copy(ysb[:, 0:512], ps)
                                else:
                                    nc.vector.tensor_copy(ysb[:, 512:1024], ps)
                            eng = nc.gpsimd if dch % 2 == 0 else nc.sync
                            eng.dma_start(
                                out=d_y[:, dch * 2:dch * 2 + 2, tt * 512:(tt + 1) * 512],
                                in_=ysb.rearrange("p (c t) -> p c t", c=2),
                            )

    nc.compile()
    _cache[key] = nc
    return nc


def kernel(hidden_states, w_q, w_k, w_v, w_o, _debug=False):
    from concourse.bass_utils import run_bass_kernel_spmd

    nc = _build(debug=_debug)
    mask = _mask_rel()
    scale = np.float32(Dh ** -0.5)

    def chunk_dmajor(w, rows, cols):
        return np.ascontiguousarray(
            w.reshape(rows, 128, cols).transpose(1, 0, 2)
        )

    in_maps = []
    for c in range(NCORES):
        b, hg = c // 4, c % 4
        hsl = slice(hg * 256, (hg + 1) * 256)
        xT = np.asarray(hidden_states[b]).T.astype(bfloat16)  # [D, S]
        in_maps.append({
            "xT": chunk_dmajor(xT, 8, S),
            "wq": chunk_dmajor((np.asarray(w_q[:, hsl]) * scale).astype(bfloat16), 8, 256),
            "wk": chunk_dmajor(np.asarray(w_k[:, hsl]).astype(bfloat16), 8, 256),
            "wv": chunk_dmajor(np.asarray(w_v[:, hsl]).astype(bfloat16), 8, 256),
            "wo": chunk_dmajor(np.asarray(w_o[hsl, :]).astype(bfloat16), 2, 1024),
            "maskT": mask,
        })

    res = run_bass_kernel_spmd(nc, in_maps, list(range(NCORES)))
    _cache["last_results"] = res

    y = np.zeros((B, S, D), np.float32)
    for c in range(NCORES):
        yT = np.asarray(res.results[c]["yT"]).astype(np.float32)  # [128, 8, S]
        y[c // 4] += yT.transpose(1, 0, 2).reshape(D, S).T
    return y



# revision 32
# speedup vs baseline: 1.0187x; 1.0187x over previous
"""Dilated attention (segment 64, dilation 4, 16 heads, head_dim 64) on 8 trn2 cores.

Sharding: 2 batches x 4 head-groups (4 heads each) = 8 cores. Each core computes
q/k/v projections for its 4 heads on its batch, block-sparse attention over the
+-2 block (256-token) dilated band, and a partial output projection. Host sums
the 4 head-group partials per batch.

Layout is fully "transposed" on-core to avoid PE transposes:
  xT   [D, S]    (D on partitions, 8 chunks of 128)
  qT/kT [64h, S] per head (head dim on partitions)
  v    [S, 64]   natural (keys on partitions) + ones column -> softmax denoms
  scoresT [k-block 128, q-window <=640] = kT_blk-stationary x qT-window
  exp (no max subtraction; scores are N(0,1)-ish), binary mask multiply,
  PV accumulates outT [65, S-half] per head in PSUM (row 64 = denominators),
  normalize via reciprocal + gpsimd partition_broadcast + DVE mul,
  yT [D, S] = wo-stationary x outT_norm, summed on host.

Scheduling strategy: attention is ACT(exp)-paced, so the k/v/q1/k1
projections are emitted BETWEEN attention stages at lower scheduler
priority — the PE fills attention dependency stalls with projection
matmuls, which also keeps the HAM clock-gate warm.
"""

import numpy as np
import ml_dtypes

bfloat16 = ml_dtypes.bfloat16

B, S, D = 2, 2048, 1024
H, Dh = 16, 64
NCORES = 8
NKB = S // 128  # 16 key blocks
WMAX = 640

_cache = {}


def _mask_rel():
    kp = np.arange(128)[:, None]
    j = np.arange(WMAX)[None, :]
    qrel = j - 256
    diff = np.abs(qrel - kp)
    seg = (qrel // 64) == (kp // 64)
    dil = (diff > 0) & (diff % 4 == 0) & (diff <= 256)
    return np.ascontiguousarray((seg | dil).astype(bfloat16))


def _win(kb):
    return max(0, kb - 2) * 128, min(NKB, kb + 3) * 128


def _pv_pieces(kb):
    """Split PV matmul for key-block kb at psum bank boundaries.

    PSUM start_tensor_calc zeroes the whole 2KB bank (zero-region), so
    start/stop are decided per BANK: the first key-block whose window
    touches a bank opens it (start=True zeroes it), the last closes it."""
    q0, q1 = _win(kb)
    out = []
    a = q0
    while a < q1:
        b = min(q1, (a // 512 + 1) * 512)
        bank = a // 512
        st = kb == max(0, 4 * bank - 2)
        sp = kb == min(NKB - 1, 4 * bank + 5)
        out.append((a, b, st, sp))
        a = b
    return q0, q1, out


def _build(debug=False):
    key = ("nc", debug)
    if key in _cache:
        return _cache[key]
    import concourse.mybir as mybir
    from concourse import bacc
    from concourse.tile import TileContext

    bf = mybir.dt.bfloat16
    f32 = mybir.dt.float32
    EXP = mybir.ActivationFunctionType.Exp

    nc = bacc.Bacc()
    d_x = nc.declare_dram_parameter("xT", [128, 8, S], bf, isOutput=False)
    d_wq = nc.declare_dram_parameter("wq", [128, 8, 256], bf, isOutput=False)
    d_wk = nc.declare_dram_parameter("wk", [128, 8, 256], bf, isOutput=False)
    d_wv = nc.declare_dram_parameter("wv", [128, 8, 256], bf, isOutput=False)
    d_wo = nc.declare_dram_parameter("wo", [128, 2, 1024], bf, isOutput=False)
    d_mask = nc.declare_dram_parameter("maskT", [128, WMAX], bf, isOutput=False)
    f16 = mybir.dt.float16
    d_y = nc.declare_dram_parameter("yT", [128, 8, S], f16, isOutput=True)
    if debug:
        d_dbg_q = nc.declare_dram_parameter("dbg_q", [128, S], bf, isOutput=True)
        d_dbg_k = nc.declare_dram_parameter("dbg_k", [128, S], bf, isOutput=True)
        d_dbg_v = nc.declare_dram_parameter("dbg_v", [128, 16 * 4 * 65], bf, isOutput=True)
        d_dbg_at = nc.declare_dram_parameter("dbg_at", [128, WMAX], bf, isOutput=True)
        d_dbg_den = nc.declare_dram_parameter("dbg_den", [4, S], f32, isOutput=True)
        d_dbg_rec = nc.declare_dram_parameter("dbg_rec", [4, S], f32, isOutput=True)
        d_dbg_on = nc.declare_dram_parameter("dbg_on", [128, S], bf, isOutput=True)

    with TileContext(nc) as tc:
        with (
            tc.tile_pool(name="const", bufs=1) as cpool,
            tc.tile_pool(name="attn", bufs=68) as apool,
            tc.tile_pool(name="ysb", bufs=4) as ypool,
            tc.tile_pool(name="small", bufs=2) as spool,
        ):
            # ---- input DMAs, dispatch spread over sync/gpsimd/scalar ----
            sb_wq = cpool.tile([128, 8, 256], bf, name="wq", tag="wq")
            sb_wk = cpool.tile([128, 8, 256], bf, name="wk", tag="wk")
            sb_wv = cpool.tile([128, 8, 256], bf, name="wv", tag="wv")
            nc.scalar.dma_start(out=sb_wq[:, 0, :], in_=d_wq[:, 0, :])
            sb_x = []
            for dc in range(8):
                t = cpool.tile([128, S], bf, name=f"x{dc}", tag=f"x{dc}")
                nc.sync.dma_start(out=t[0:64, :], in_=d_x[0:64, dc, :])
                nc.gpsimd.dma_start(out=t[64:128, :], in_=d_x[64:128, dc, :])
                sb_x.append(t)
            nc.scalar.dma_start(out=sb_wq[:, 1:8, :], in_=d_wq[:, 1:8, :])
            nc.scalar.dma_start(out=sb_wk[:, :, :], in_=d_wk[:, :, :])
            nc.scalar.dma_start(out=sb_wv[:, :, :], in_=d_wv[:, :, :])
            sb_wo = cpool.tile([128, 2, 1024], bf, name="wo", tag="wo")
            nc.scalar.dma_start(out=sb_wo, in_=d_wo[:, :, :])
            sb_mask = cpool.tile([128, WMAX], bf, name="mask", tag="mask")
            nc.scalar.dma_start(out=sb_mask, in_=d_mask[:, :])

            sb_q = []
            sb_k = []
            sb_on = []
            for p in range(2):
                sb_q.append(cpool.tile([128, S], bf, name=f"q{p}", tag=f"q{p}"))
                sb_k.append(cpool.tile([128, S], bf, name=f"k{p}", tag=f"k{p}"))
                sb_on.append(cpool.tile([128, S], bf, name=f"on{p}", tag=f"on{p}"))
            sb_v = cpool.tile([128, 16, 4, 65], bf, name="v", tag="v")
            nc.vector.memset(sb_v[:, :, :, 64:65], 1.0)

            # ---- era 0: q0 + k0 projections, dc-outer accumulators.
            # Dep-free junk matmuls interleave into the dc chains so the HAM
            # busy-window stays fed during x-chunk DMA arrival gaps (else the
            # PE clock halves at ~14us and the whole era runs 2x slow).
            wpool = tc.alloc_tile_pool(name="warm", bufs=1, space="PSUM")
            junk = cpool.tile([128, 512], bf, name="junk", tag="junk")
            nc.vector.memset(junk, 0.0)
            wps = wpool.tile([128, 512], f32, name="wps", tag="wps")

            def junk_mm(n=1):
                for _ in range(n):
                    nc.tensor.matmul(wps, lhsT=junk[:, 0:128], rhs=junk,
                                     start=True, stop=True)

            with tc.tile_pool(name="pjq", bufs=1, space="PSUM") as pjq:
                junk_mm(12)
                for w_sb, dst, scope in (
                    (sb_wq, sb_q, "proj_q0"),
                    (sb_wk, sb_k, "proj_k0"),
                ):
                    with nc.named_scope(scope):
                        acc = {}
                        for tt in range(4):
                            acc[tt] = pjq.tile([128, 512], f32, name=f"a{tt}", tag=f"aq{tt}")
                        for dc in range(8):
                            for tt in range(4):
                                nc.tensor.matmul(
                                    acc[tt],
                                    lhsT=w_sb[:, dc, 0:128],
                                    rhs=sb_x[dc][:, tt * 512:(tt + 1) * 512],
                                    start=(dc == 0),
                                    stop=(dc == 7),
                                )
                            if scope == "proj_q0":
                                junk_mm(2)
                        for tt in range(4):
                            nc.vector.tensor_copy(dst[0][:, tt * 512:(tt + 1) * 512], acc[tt])
            wpool.release()

            # ---- attention era: sc(2x2) + ot(3x1) + pj(1x1) = 8 psum banks ----
            with (
                tc.tile_pool(name="sc", bufs=2, space="PSUM") as scp,
                tc.tile_pool(name="ot", bufs=3, space="PSUM") as otp,
                tc.tile_pool(name="pj", bufs=1, space="PSUM") as pj,
            ):
                def proj_qk(w_sb, dst, p, scope):
                    with nc.named_scope(scope):
                        for tt in range(4):
                            ps = pj.tile([128, 512], f32, name="pspj", tag="pj")
                            for dc in range(8):
                                nc.tensor.matmul(
                                    ps,
                                    lhsT=w_sb[:, dc, p * 128:(p + 1) * 128],
                                    rhs=sb_x[dc][:, tt * 512:(tt + 1) * 512],
                                    start=(dc == 0),
                                    stop=(dc == 7),
                                )
                            nc.vector.tensor_copy(dst[p][:, tt * 512:(tt + 1) * 512], ps)

                def proj_v():
                    with nc.named_scope("proj_v"):
                        for t in range(16):
                            ps = pj.tile([128, 256], f32, name="psv", tag="pj")
                            for dc in range(8):
                                nc.tensor.matmul(
                                    ps,
                                    lhsT=sb_x[dc][:, t * 128:(t + 1) * 128],
                                    rhs=sb_wv[:, dc, :],
                                    start=(dc == 0),
                                    stop=(dc == 7),
                                )
                            nc.vector.tensor_copy(
                                sb_v[:, t, :, 0:64],
                                ps.rearrange("p (h d) -> p h d", h=4),
                            )

                def scores_phase(p):
                    ats = {}
                    with nc.named_scope(f"scores_p{p}"):
                        for kb in range(NKB):
                            q0, q1 = _win(kb)
                            wk_ = q1 - q0
                            j0 = q0 - (kb - 2) * 128
                            # two sc tiles (distinct psum slots) so the two
                            # K=64 head matmuls pack into row-groups 0-1/2-3
                            # of the PE array and run concurrently
                            sc = [
                                scp.tile([128, WMAX], f32, name=f"sc{hh}", tag="sc")
                                for hh in range(2)
                            ]
                            a = 0
                            while a < wk_:
                                b = min(a + 512, wk_)
                                for hh in range(2):
                                    half = hh * 64
                                    nc.tensor.matmul(
                                        sc[hh][:, a:b],
                                        lhsT=sb_k[p][half:half + 64, kb * 128:(kb + 1) * 128],
                                        rhs=sb_q[p][half:half + 64, q0 + a:q0 + b],
                                        start=True,
                                        stop=True,
                                    )
                                a = b
                            for hh in range(2):
                                at = apool.tile([128, WMAX], bf, name="at", tag="at")
                                nc.scalar.activation(at[:, :wk_], sc[hh][:, :wk_], EXP)
                                nc.vector.tensor_mul(
                                    at[:, :wk_], at[:, :wk_], sb_mask[:, j0:j0 + wk_]
                                )
                                ats[hh, kb] = at
                                if debug and p == 0 and hh == 0 and kb == 8:
                                    nc.sync.dma_start(out=d_dbg_at[:, :], in_=at[:, :])
                    return ats

                def pv_phase(p, hhs, ats):
                    bank_tiles = {}

                    def normalize(hh, bk, ot):
                        h = 2 * p + hh
                        half = hh * 64
                        base = bk * 512
                        den = spool.tile([1, 512], f32, name="den", tag="den")
                        nc.scalar.copy(den, ot[64:65, :])
                        rec = spool.tile([1, 512], f32, name="rec", tag="rec")
                        nc.vector.reciprocal_approx_fast(rec, den)
                        bc = spool.tile([64, 512], f32, name="bc", tag="bc")
                        nc.gpsimd.partition_broadcast(bc, rec)
                        if debug:
                            nc.sync.dma_start(out=d_dbg_rec[h:h + 1, base:base + 512], in_=rec)
                        nc.vector.tensor_mul(
                            sb_on[p][half:half + 64, base:base + 512],
                            ot[0:64, :], bc,
                        )

                    with nc.named_scope(f"pv_p{p}"):
                        for kb in range(NKB):
                            q0, q1, pieces = _pv_pieces(kb)
                            for hh in hhs:
                                h = 2 * p + hh
                                at = ats[hh, kb]
                                vv = sb_v[:, kb, h, :]
                                for a, b, st, sp_ in pieces:
                                    bk = a // 512
                                    if st:
                                        bank_tiles[hh, bk] = otp.tile(
                                            [65, 512], f32, name=f"o{h}b{bk}", tag="outp"
                                        )
                                    nc.tensor.matmul(
                                        bank_tiles[hh, bk][:, a - bk * 512:b - bk * 512],
                                        lhsT=vv,
                                        rhs=at[:, a - q0:b - q0],
                                        start=st,
                                        stop=sp_,
                                    )
                                    if sp_:
                                        normalize(hh, bk, bank_tiles[hh, bk])

                # pair 0: scores (ACT-paced), v-proj fills PE stalls
                ats0 = scores_phase(0)
                proj_v()
                # pair-1 projections early so scores1 can pipeline during pv0
                proj_qk(sb_wq, sb_q, 1, "proj_q1")
                proj_qk(sb_wk, sb_k, 1, "proj_k1")
                pv_phase(0, (0,), ats0)
                pv_phase(0, (1,), ats0)
                ats1 = scores_phase(1)
                pv_phase(1, (0, 1), ats1)

                if debug:
                    nc.sync.dma_start(out=d_dbg_q[:, :], in_=sb_q[0][:, :])
                    nc.sync.dma_start(out=d_dbg_k[:, :], in_=sb_k[0][:, :])
                    nc.sync.dma_start(
                        out=d_dbg_v[:, :],
                        in_=sb_v.rearrange("p a b c -> p (a b c)"),
                    )
                    nc.sync.dma_start(out=d_dbg_on[:, :], in_=sb_on[0][:, :])

                # ---- output projection: yT = wo^T @ outT_norm ----
                # shares the ot psum pool so early tt-waves overlap the tail
                with nc.named_scope("proj_y"):
                    f16_ = mybir.dt.float16
                    for tt in range(4):
                        for dch in range(4):
                            ysb = ypool.tile([128, 1024], f16_, name="ysb", tag="ysb")
                            for sub in range(2):
                                dc = dch * 2 + sub
                                ps = otp.tile([128, 512], f32, name="psy", tag="outp")
                                for kc in range(2):
                                    nc.tensor.matmul(
                                        ps,
                                        lhsT=sb_wo[:, kc, dc * 128:(dc + 1) * 128],
                                        rhs=sb_on[kc][:, tt * 512:(tt + 1) * 512],
                                        start=(kc == 0),
                                        stop=(kc == 1),
                                    )
                                if sub == 0:
                                    nc.scalar.copy(ysb[:, 0:512], ps)
                                else:
                                    nc.vector.tensor_copy(ysb[:, 512:1024], ps)
                            eng = nc.gpsimd if dch % 2 == 0 else nc.sync
                            eng.dma_start(
                                out=d_y[:, dch * 2:dch * 2 + 2, tt * 512:(tt + 1) * 512],
                                in_=ysb.rearrange("p (c t) -> p c t", c=2),
                            )

    nc.compile()
    _cache[key] = nc
    return nc


def kernel(hidden_states, w_q, w_k, w_v, w_o, _debug=False):
    from concourse.bass_utils import run_bass_kernel_spmd

    nc = _build(debug=_debug)
    mask = _mask_rel()
    scale = np.float32(Dh ** -0.5)

    def chunk_dmajor(w, rows, cols):
        return np.ascontiguousarray(
            w.reshape(rows, 128, cols).transpose(1, 0, 2)
        )

    in_maps = []
    for c in range(NCORES):
        b, hg = c // 4, c % 4
        hsl = slice(hg * 256, (hg + 1) * 256)
        xT = np.asarray(hidden_states[b]).T.astype(bfloat16)  # [D, S]
        in_maps.append({
            "xT": chunk_dmajor(xT, 8, S),
            "wq": chunk_dmajor((np.asarray(w_q[:, hsl]) * scale).astype(bfloat16), 8, 256),
            "wk": chunk_dmajor(np.asarray(w_k[:, hsl]).astype(bfloat16), 8, 256),
            "wv": chunk_dmajor(np.asarray(w_v[:, hsl]).astype(bfloat16), 8, 256),
            "wo": chunk_dmajor(np.asarray(w_o[hsl, :]).astype(bfloat16), 2, 1024),
            "maskT": mask,
        })

    res = run_bass_kernel_spmd(nc, in_maps, list(range(NCORES)))
    _cache["last_results"] = res

    y = np.zeros((B, S, D), np.float32)
    for c in range(NCORES):
        yT = np.asarray(res.results[c]["yT"]).astype(np.float32)  # [128, 8, S]
        y[c // 4] += yT.transpose(1, 0, 2).reshape(D, S).T
    return y

